# Initial kernel scaffold
#
"""BFConv2d Trainium2 kernel.

Reference computation (per problem spec):
  xq = bfp_quantize(x)        # 8-bit mantissa, shared exp per 32 channels
  wq = bfp_quantize(weight)   # groups along Cin
  out = conv2d(xq, wq, stride 1, pad 1) + bias
  out = bfp_quantize(out)     # groups along Cout

Sharding: data-parallel over batch B=32 -> 4 images per core x 8 cores.
Weight/bias replicated; no collectives.

Per-core kernel design:
  - BFP quantization uses the DVE StreamTranspose (32x32 block transpose;
    group size == 32) to get channel groups onto the free axis, a
    reduce_max(abs) for the group absmax, int32 bit tricks for the shared
    exponent, and a magic-number add/sub (T = 1.5*2^(e+16)) for exact
    round-half-even to the BFP grid.  clip via max/min against
    LO_T = T - 2^e, HI_T = T + (127/128)*2^e (exact in fp32).
  - Quantized x/w are exactly representable in bf16 -> conv runs as bf16
    matmuls (18 per psum tile: 2 ci-halves x 9 taps) accumulating in PSUM.
  - x is stored zero-padded [(H+2)*(W+2)] so conv taps are AP shifts.
  - Output: ACT copies PSUM->SBUF fused with bias add, then the same BFP
    quantize pipeline, DMA out.
"""

import os
import sys

sys.path.insert(0, "/opt/trn_rl_repo")

import numpy as np

import concourse.bass as bass
import concourse.mybir as mybir
import concourse.tile as tile

F32 = mybir.dt.float32
I32 = mybir.dt.int32
BF16 = mybir.dt.bfloat16
AX = mybir.AxisListType
OP = mybir.AluOpType

N_CORES = 8
B, CIN, H, W = 32, 256, 56, 56
COUT, KK = 256, 3
B_LOCAL = B // N_CORES


def _emit_quant(nc, pool, src_ap, dst_ap, npix, tag, eng_tt):
    """Emit BFP-quantize of src_ap [128, npix] f32 -> dst_ap (any layout,
    free size npix, dtype of dst tile).  Groups = 32 consecutive partitions.

    eng_tt: list of 4 engines for the y1..y4 tensor_tensor passes.
    """
    P = 128
    nj = npix // 32
    x_bt = pool.tile([P, npix], F32, tag=tag + "w")
    nc.vector.transpose(x_bt[:], src_ap)

    A = pool.tile([P, nj], F32, tag=tag + "A")
    nc.vector.tensor_reduce(
        A[:], x_bt[:].rearrange("p (j f) -> p j f", f=32),
        axis=AX.X, op=OP.max, apply_absolute_value=True,
    )
    Ai = A[:].bitcast(I32)
    nc.vector.tensor_scalar(Ai, Ai, 0x7F800000, 0x00800000,
                            OP.bitwise_and, OP.max)
    T = pool.tile([P, nj], F32, tag=tag + "T")
    nc.vector.tensor_scalar(T[:].bitcast(I32), Ai, 0x08400000, None, OP.add)
    LO = pool.tile([P, nj], F32, tag=tag + "L")
    nc.vector.tensor_sub(LO[:], T[:], A[:])
    HI = pool.tile([P, nj], F32, tag=tag + "H")
    nc.vector.tensor_scalar(HI[:].bitcast(I32), Ai, 0x00020000, None,
                            OP.subtract)
    nc.vector.tensor_add(HI[:], T[:], HI[:])

    def bc(t):
        return t[:, :, None].to_broadcast((P, nj, 32))

    def v3(t):
        return t[:].rearrange("p (j f) -> p j f", f=32)

    y1 = pool.tile([P, npix], F32, tag=tag + "w")
    eng_tt[0].tensor_tensor(v3(y1), v3(x_bt), bc(T), OP.add)
    y2 = pool.tile([P, npix], F32, tag=tag + "w")
    eng_tt[1].tensor_tensor(v3(y2), v3(y1), bc(LO), OP.max)
    y3 = pool.tile([P, npix], F32, tag=tag + "w")
    eng_tt[2].tensor_tensor(v3(y3), v3(y2), bc(HI), OP.min)
    y4 = pool.tile([P, npix], dst_ap.dtype, tag=tag + "q")
    eng_tt[3].tensor_tensor(v3(y4), v3(y3), bc(T), OP.subtract)

    nc.vector.transpose(dst_ap, y4[:])


def build_kernel(b_local=B_LOCAL, h=H, w=W):
    """Build the per-core Bass module."""
    nc = bass.Bass("TRN2")
    P = 128
    hw = h * w
    hp, wp = h + 2, w + 2
    hwp = hp * wp

    x_in = nc.dram_tensor("x", [b_local, CIN, h, w], F32, kind="ExternalInput")
    w_in = nc.dram_tensor("weight", [COUT, CIN, KK, KK], F32,
                          kind="ExternalInput")
    b_in = nc.dram_tensor("bias", [COUT], F32, kind="ExternalInput")
    o_out = nc.dram_tensor("out", [b_local, COUT, h, w], F32,
                           kind="ExternalOutput")

    # conv output tiling: rows per psum tile
    rpt = max(1, 448 // w)
    while h % rpt:
        rpt -= 1
    n_pt = h // rpt
    npix_t = rpt * w           # free size per psum tile (<= 448)

    # x-quant chunking: rows per chunk such that rows*w % 32 == 0, ~<=1600
    xc_rows = max(1, 1568 // w)
    while h % xc_rows or (xc_rows * w) % 32:
        xc_rows -= 1
    n_xc = h // xc_rows
    xc_pix = xc_rows * w

    # out-quant chunking: groups of psum tiles, pix % 32 == 0
    out_chunks = []  # list of (start_pt, n_pts)
    acc = 0
    start = 0
    for pt in range(n_pt):
        acc += npix_t
        if acc % 32 == 0 and (acc >= 1568 or pt == n_pt - 1):
            out_chunks.append((start, acc))
            start = pt + 1
            acc = 0
    assert acc == 0, "out chunking failed"

    with tile.TileContext(nc) as tc:
        # ---------------- weight prep ----------------
        with tc.tile_pool(name="wstart", bufs=1) as wsp, \
             tc.tile_pool(name="wpsum", bufs=2, space="PSUM") as wpp, \
             tc.tile_pool(name="persist", bufs=1) as pp:

            ident = pp.tile([P, P], BF16, tag="ident")
            from concourse.masks import make_identity
            make_identity(nc, ident[:])

            bias_sb = pp.tile([P, 2], F32, tag="bias")
            for ch in range(2):
                nc.sync.dma_start(bias_sb[:, ch:ch + 1],
                                  b_in[ch * P:(ch + 1) * P, None])

            # lhsT[ci_half]: [128 ci, 9*256] bf16, free idx = khw*256 + co
            lhsT = [pp.tile([P, 9 * COUT], BF16, tag=f"lhsT{i}")
                    for i in range(2)]

            for co_half in range(2):
                w_nat = wsp.tile([P, CIN * 9], F32, tag="wnat")
                nc.sync.dma_start(
                    w_nat[:],
                    w_in[co_half * P:(co_half + 1) * P].rearrange(
                        "o i kh kw -> o (i kh kw)"))

                # quantize along ci groups (free axis): view [p, g, c, k]
                ng = CIN // 32
                A = wsp.tile([P, ng * 9], F32, tag="wA")
                nc.vector.tensor_reduce(
                    A[:].rearrange("p (g k) -> p g k", g=ng),
                    w_nat[:].rearrange("p (g c k) -> p g k c", g=ng, c=32),
                    axis=AX.X, op=OP.max, apply_absolute_value=True,
                )
                Ai = A[:].bitcast(I32)
                nc.vector.tensor_scalar(Ai, Ai, 0x7F800000, 0x00800000,
                                        OP.bitwise_and, OP.max)
                T = wsp.tile([P, ng * 9], F32, tag="wT")
                nc.vector.tensor_scalar(T[:].bitcast(I32), Ai, 0x08400000,
                                        None, OP.add)
                LO = wsp.tile([P, ng * 9], F32, tag="wL")
                nc.vector.tensor_sub(LO[:], T[:], A[:])
                HI = wsp.tile([P, ng * 9], F32, tag="wH")
                nc.vector.tensor_scalar(HI[:].bitcast(I32), Ai, 0x00020000,
                                        None, OP.subtract)
                nc.vector.tensor_add(HI[:], T[:], HI[:])

                def wv(t):
                    return t[:].rearrange("p (g c k) -> p g c k", g=ng, c=32)

                def wb(t):
                    return t[:].rearrange("p (g k) -> p g k", g=ng)[
                        :, :, None, :].to_broadcast((P, ng, 32, 9))

                y1 = wsp.tile([P, CIN * 9], F32, tag="wy1")
                nc.gpsimd.tensor_tensor(wv(y1), wv(w_nat), wb(T), OP.add)
                y2 = wsp.tile([P, CIN * 9], F32, tag="wy2")
                nc.gpsimd.tensor_tensor(wv(y2), wv(y1), wb(LO), OP.max)
                y3 = wsp.tile([P, CIN * 9], F32, tag="wy1")
                nc.vector.tensor_tensor(wv(y3), wv(y2), wb(HI), OP.min)
                wq = wsp.tile([P, CIN * 9], BF16, tag="wq")
                nc.vector.tensor_tensor(wv(wq), wv(y3), wb(T), OP.subtract)

                # transpose to lhsT: for each (ci_half, khw):
                #   [co 128, ci 128 (stride 9)] -> [ci, co]
                for ci_half in range(2):
                    for khw in range(9):
                        tp = wpp.tile([P, P], F32, tag="wtp")
                        src = bass.AP(
                            tensor=wq[:].tensor,
                            offset=wq[:].offset + 2 * (ci_half * P * 9 + khw),
                            ap=[list(p) for p in zip(
                                [wq[:].ap[0][0], 9 * 2], [P, P])],
                        )
                        src = wq[:, ci_half * P * 9 + khw:
                                 ci_half * P * 9 + khw + 9 * P:9]
                        nc.tensor.transpose(tp[:], src, ident[:])
                        nc.scalar.copy(
                            lhsT[ci_half][:, khw * COUT + co_half * P:
                                          khw * COUT + co_half * P + P],
                            tp[:])

            # ---------------- main pipeline ----------------
            with tc.tile_pool(name="xq", bufs=2) as xqp, \
                 tc.tile_pool(name="xs", bufs=2) as xsp, \
                 tc.tile_pool(name="xw", bufs=4) as xwp, \
                 tc.tile_pool(name="os", bufs=2) as osp, \
                 tc.tile_pool(name="ow", bufs=4) as owp, \
                 tc.tile_pool(name="cpsum", bufs=4, space="PSUM") as cpp:

                for img in range(b_local):
                    # ---- x quantize (both ci halves) ----
                    xq_pad = []
                    for ci_half in range(2):
                        x_nat = xsp.tile([P, hw], F32, tag="xnat")
                        nc.sync.dma_start(
                            x_nat[:],
                            x_in[img, ci_half * P:(ci_half + 1) * P].rearrange(
                                "c h w -> c (h w)"))
                        qpad = xqp.tile([P, hwp], BF16, tag=f"qpad{ci_half}")
                        # zero borders: top+bottom rows, left+right cols
                        nc.gpsimd.memset(
                            qpad[:].rearrange("p (r c) -> p r c", r=hp)[
                                :, 0:hp:hp - 1, :], 0.0)
                        nc.gpsimd.memset(
                            qpad[:].rearrange("p (r c) -> p r c", r=hp)[
                                :, :, 0:wp:wp - 1], 0.0)
                        for xc in range(n_xc):
                            src = x_nat[:, xc * xc_pix:(xc + 1) * xc_pix]
                            dst = qpad[:].rearrange(
                                "p (r c) -> p r c", r=hp)[
                                :, xc * xc_rows + 1:xc * xc_rows + 1 + xc_rows,
                                1:1 + w]
                            _emit_quant(nc, xwp, src, dst, xc_pix, "x",
                                        [nc.gpsimd, nc.gpsimd,
                                         nc.vector, nc.vector])
                        xq_pad.append(qpad)

                    # ---- conv + out quantize ----
                    for co_half in range(2):
                        o_nat = osp.tile([P, hw], F32, tag="onat")
                        for pt in range(n_pt):
                            ps = cpp.tile([P, npix_t], F32, tag="cps")
                            k = 0
                            for ci_half in range(2):
                                for kh in range(3):
                                    for kw in range(3):
                                        khw = kh * 3 + kw
                                        rhs = xq_pad[ci_half][:].rearrange(
                                            "p (r c) -> p r c", r=hp)[
                                            :, pt * rpt + kh:
                                            pt * rpt + kh + rpt,
                                            kw:kw + w]
                                        nc.tensor.matmul(
                                            ps[:].rearrange(
                                                "p (r c) -> p r c", r=rpt),
                                            lhsT[ci_half][
                                                :, khw * COUT + co_half * P:
                                                khw * COUT + co_half * P + P],
                                            rhs,
                                            start=(k == 0), stop=(k == 17))
                                        k += 1
                            nc.scalar.activation(
                                o_nat[:, pt * npix_t:(pt + 1) * npix_t],
                                ps[:],
                                mybir.ActivationFunctionType.Identity,
                                bias=bias_sb[:, co_half:co_half + 1])

                        for (spt, cpix) in out_chunks:
                            src = o_nat[:, spt * npix_t:spt * npix_t + cpix]
                            oq = osp.tile([P, cpix], F32, tag="oq")
                            _emit_quant(nc, owp, src, oq[:], cpix, "o",
                                        [nc.gpsimd, nc.gpsimd,
                                         nc.vector, nc.vector])
                            nc.sync.dma_start(
                                o_out[img,
                                      co_half * P:(co_half + 1) * P].rearrange(
                                    "c h w -> c (h w)")[
                                    :, spt * npix_t:spt * npix_t + cpix],
                                oq[:])
    return nc


_NC_CACHE = {}


def _get_nc(key):
    if key not in _NC_CACHE:
        _NC_CACHE[key] = build_kernel(*key)
    return _NC_CACHE[key]


def kernel(x, weight, bias):
    from concourse import bass_utils

    nc = _get_nc((B_LOCAL, H, W))
    in_maps = []
    for core in range(N_CORES):
        in_maps.append({
            "x": np.ascontiguousarray(x[core * B_LOCAL:(core + 1) * B_LOCAL]),
            "weight": np.ascontiguousarray(weight),
            "bias": np.ascontiguousarray(bias),
        })
    res = bass_utils.run_bass_kernel_spmd(
        nc, in_maps, core_ids=list(range(N_CORES)),
        trace=bool(int(os.environ.get("BFC_TRACE", "0"))),
    )
    out = np.concatenate([r["out"] for r in res.results], axis=0)
    if res.exec_time_ns is not None:
        kernel.last_exec_time_ns = res.exec_time_ns
        kernel.last_mean_exec_time_ns = res.mean_exec_time_ns
        kernel.last_trace = res.instructions_and_trace
    return out


kernel.last_exec_time_ns = None


# revision 13
# speedup vs baseline: 1.0753x; 1.0753x over previous
"""BFConv2d Trainium2 kernel.

Reference computation (per problem spec):
  xq = bfp_quantize(x)        # 8-bit mantissa, shared exp per 32 channels
  wq = bfp_quantize(weight)   # groups along Cin
  out = conv2d(xq, wq, stride 1, pad 1) + bias
  out = bfp_quantize(out)     # groups along Cout

Sharding: data-parallel over batch B=32 -> 4 images per core x 8 cores.
Weight/bias replicated; no collectives.

Per-core kernel design:
  - BFP quantization uses the DVE StreamTranspose (32x32 block transpose;
    group size == 32) to get channel groups onto the free axis, a
    reduce_max(abs) for the group absmax, int32 bit tricks for the shared
    exponent, and a magic-number add/sub (T = 1.5*2^(e+16)) for exact
    round-half-even to the BFP grid.  clip via max/min against
    LO_T = T - 2^e, HI_T = T + (127/128)*2^e (exact in fp32).
  - Quantized x/w are exactly representable in bf16 -> conv runs as bf16
    matmuls (18 per psum tile: 2 ci-halves x 9 taps) accumulating in PSUM.
  - x is stored zero-padded [(H+2)*(W+2)] so conv taps are AP shifts.
  - Output: ACT copies PSUM->SBUF fused with bias add, then the same BFP
    quantize pipeline, DMA out.
"""

import os
import sys

sys.path.insert(0, "/opt/trn_rl_repo")

import numpy as np

import concourse.bass as bass
import concourse.mybir as mybir
import concourse.tile as tile
from concourse import bacc

F32 = mybir.dt.float32
I32 = mybir.dt.int32
BF16 = mybir.dt.bfloat16
AX = mybir.AxisListType
OP = mybir.AluOpType

N_CORES = 8
B, CIN, H, W = 32, 256, 56, 56
COUT, KK = 256, 3
B_LOCAL = B // N_CORES
P = 128


# Quantize-in-normalized-domain constants.  After u = x * 2^-e (exact),
# round-to-1/128-grid via magic add T0 = 1.5*2^16; clip to [-1, 127/128].
T0 = 98304.0            # 1.5 * 2^16
LO_T0 = 98303.0         # T0 - 1
HI_T0 = 98304.9921875   # T0 + 127/128  (exact in fp32)


def _emit_quant_group(nc, pool, chunks, tag, dst_dtype, consts):
    ct_inv, bias_T0, bias_nLO = consts
    """BFP-quantize a half-image: chunks = [(src_ap [128, npix], dst_ap)].
    Groups = 32 consecutive partitions (block-transposed via StreamTranspose).

    Per chunk: ST -> absmax-reduce -> (consolidated exponent ops) ->
    u = x*2^-e [gpsimd] -> ts(+T0, max LO_T0) -> ts(min HI_T0, -T0) [DVE 2x]
    -> q = u*2^e [gpsimd, casts to dst dtype] -> ST-back.
    ct_inv: const int32 tile [128,1] holding (254<<23) for inverse exponent.
    """
    njs = [src.shape[-1] // 32 for src, _ in chunks]
    nj_tot = sum(njs)
    A = pool.tile([P, nj_tot], F32, tag=tag + "A")
    x_bts = []
    off = 0
    for (src, _), nj in zip(chunks, njs):
        npix = nj * 32
        x_bt = pool.tile([P, npix], F32, tag=tag + "w", name="x_bt")
        nc.vector.transpose(x_bt[:], src)
        nc.vector.tensor_reduce(
            A[:, off:off + nj],
            x_bt[:].rearrange("p (j f) -> p j f", f=32),
            axis=AX.X, op=OP.max, apply_absolute_value=True,
        )
        x_bts.append(x_bt)
        off += nj

    # E = 2^floor(log2(absmax)) (guarded >= 2^-126); invE = 2^-floor(...)
    Ai = A[:].bitcast(I32)
    nc.vector.tensor_single_scalar(Ai, Ai, 0x7F800000, OP.bitwise_and)
    nc.vector.tensor_single_scalar(Ai, Ai, 0x00800000, OP.max)
    invE = pool.tile([P, nj_tot], F32, tag=tag + "I")
    nc.vector.tensor_tensor(
        invE[:].bitcast(I32),
        ct_inv[:, 0:1].to_broadcast((P, nj_tot)), Ai,
        OP.subtract)

    off = 0
    for (src, dst), nj, x_bt in zip(chunks, njs, x_bts):
        npix = nj * 32

        def bcs(t):
            return t[:, off:off + nj, None].to_broadcast((P, nj, 32))

        def v3(t):
            return t[:].rearrange("p (j f) -> p j f", f=32)

        u = pool.tile([P, npix], F32, tag=tag + "w", name="u")
        nc.gpsimd.tensor_tensor(v3(u), v3(x_bt), bcs(invE), OP.mult)
        # v = RNE(u + T0): the magic round (ACT Identity bias-add, exact)
        y = pool.tile([P, npix], F32, tag=tag + "w", name="y")
        nc.scalar.activation(y[:], u[:],
                             mybir.ActivationFunctionType.Identity,
                             bias=bias_T0[:, 0:1])
        # w1 = max(v - LO_T0, 0)  (clip low; exact Sterbenz subtract)
        nc.scalar.activation(y[:], y[:],
                             mybir.ActivationFunctionType.Relu,
                             bias=bias_nLO[:, 0:1])
        # z = min(w1, 255/128) - 1  (clip high + undo offset)
        z = pool.tile([P, npix], F32, tag=tag + "w", name="z")
        nc.vector.tensor_scalar(z[:], y[:], 1.9921875, 1.0,
                                OP.min, OP.subtract)
        q = pool.tile([P, npix], dst_dtype, tag=tag + "q", name="q")
        nc.gpsimd.tensor_tensor(v3(q), v3(z), bcs(A), OP.mult)
        nc.vector.transpose(dst, q[:])
        off += nj


def build_kernel(b_local=B_LOCAL, h=H, w=W):
    """Build the per-core Bass module."""
    nc = bacc.Bacc("TRN2")
    hw = h * w
    hp, wp = h + 2, w + 2
    hwp = hp * wp

    x_in = nc.dram_tensor("x", [b_local, CIN, h, w], F32, kind="ExternalInput")
    w_in = nc.dram_tensor("weight", [COUT, CIN, KK, KK], F32,
                          kind="ExternalInput")
    b_in = nc.dram_tensor("bias", [COUT], F32, kind="ExternalInput")
    o_out = nc.dram_tensor("out", [b_local, COUT, h, w], F32,
                           kind="ExternalOutput")

    # conv output tiling: rows per psum tile (free <= 448)
    rpt = max(1, 448 // w)
    while h % rpt:
        rpt -= 1
    n_pt = h // rpt
    npix_t = rpt * w

    # x-quant chunking: rows per chunk, rows*w % 32 == 0, ~<=1600 px
    xc_rows = max(1, 1568 // w)
    while h % xc_rows or (xc_rows * w) % 32:
        xc_rows -= 1
    n_xc = h // xc_rows
    xc_pix = xc_rows * w

    # out-quant chunking: whole psum tiles, pix % 32 == 0
    out_chunks = []  # (start_pt, pix)
    acc, start = 0, 0
    for pt in range(n_pt):
        acc += npix_t
        if acc % 32 == 0 and (acc >= 1568 or pt == n_pt - 1):
            out_chunks.append((start, acc))
            start, acc = pt + 1, 0
    assert acc == 0, "out chunking failed"

    with tile.TileContext(nc) as tc:
        with tc.tile_pool(name="persist", bufs=1) as pp:
            ident = pp.tile([P, P], BF16, tag="ident")
            from concourse.masks import make_identity
            make_identity(nc, ident[:])

            ct_inv = pp.tile([P, 1], I32, tag="ctinv")
            nc.gpsimd.memset(ct_inv[:], 254 << 23)
            bias_T0 = pp.tile([P, 1], F32, tag="biasT0")
            nc.gpsimd.memset(bias_T0[:], T0)
            bias_nLO = pp.tile([P, 1], F32, tag="biasnLO")
            nc.gpsimd.memset(bias_nLO[:], -LO_T0)
            consts = (ct_inv, bias_T0, bias_nLO)

            bias_sb = pp.tile([P, 2], F32, tag="bias")
            for ch in range(2):
                nc.sync.dma_start(bias_sb[:, ch:ch + 1],
                                  b_in[ch * P:(ch + 1) * P, None])

            # lhsT[ci_half]: [128 ci, 9*256] bf16, free idx = khw*256 + co
            lhsT = [pp.tile([P, 9 * COUT], BF16, tag=f"lhsT{i}",
                            name=f"lhsT{i}") for i in range(2)]

            # ---------------- weight prep ----------------
            ng = CIN // 32
            with tc.tile_pool(name="wstart", bufs=1) as wsp, \
                 tc.tile_pool(name="wpsum", bufs=2, space="PSUM") as wpp:
                for co_half in range(2):
                    w_nat = wsp.tile([P, CIN * 9], F32, tag="wnat")
                    nc.sync.dma_start(
                        w_nat[:],
                        w_in[co_half * P:(co_half + 1) * P].rearrange(
                            "o i kh kw -> o (i kh kw)"))

                    A = wsp.tile([P, ng * 9], F32, tag="wA")
                    nc.vector.tensor_reduce(
                        A[:].rearrange("p (g k) -> p g k", g=ng),
                        w_nat[:].rearrange("p (g c k) -> p g k c", g=ng, c=32),
                        axis=AX.X, op=OP.max, apply_absolute_value=True,
                    )
                    Ai = A[:].bitcast(I32)
                    nc.vector.tensor_single_scalar(Ai, Ai, 0x7F800000,
                                                   OP.bitwise_and)
                    nc.vector.tensor_single_scalar(Ai, Ai, 0x00800000, OP.max)
                    invE = wsp.tile([P, ng * 9], F32, tag="wI")
                    nc.vector.tensor_tensor(
                        invE[:].bitcast(I32),
                        ct_inv[:, 0:1].to_broadcast((P, ng * 9)), Ai,
                        OP.subtract)

                    def wv(t):
                        return t[:].rearrange("p (g c k) -> p g c k",
                                              g=ng, c=32)

                    def wb(t):
                        return t[:].rearrange("p (g k) -> p g k", g=ng)[
                            :, :, None, :].to_broadcast((P, ng, 32, 9))

                    y1 = wsp.tile([P, CIN * 9], F32, tag="wy1")
                    nc.vector.tensor_tensor(wv(y1), wv(w_nat), wb(invE),
                                            OP.mult)
                    y2 = wsp.tile([P, CIN * 9], F32, tag="wy2")
                    nc.vector.tensor_scalar(y2[:], y1[:], T0, LO_T0,
                                            OP.add, OP.max)
                    y3 = wsp.tile([P, CIN * 9], F32, tag="wy1")
                    nc.vector.tensor_scalar(y3[:], y2[:], HI_T0, T0,
                                            OP.min, OP.subtract)
                    wq = wsp.tile([P, CIN * 9], BF16, tag="wq")
                    nc.gpsimd.tensor_tensor(wv(wq), wv(y3), wb(A), OP.mult)

                    # [co 128, ci 128 (stride 9)] -> [ci, co] per (ci_half,khw)
                    wq_kci = wq[:].rearrange("p (ci k) -> p k ci", k=9)
                    for ci_half in range(2):
                        for khw in range(9):
                            tp = wpp.tile([P, P], BF16, tag="wtp")
                            src = wq_kci[:, khw,
                                         ci_half * P:(ci_half + 1) * P]
                            nc.tensor.transpose(tp[:], src, ident[:])
                            nc.scalar.copy(
                                lhsT[ci_half][:, khw * COUT + co_half * P:
                                              khw * COUT + co_half * P + P],
                                tp[:])

            # ---------------- main pipeline ----------------
            with tc.tile_pool(name="xq", bufs=2) as xqp, \
                 tc.tile_pool(name="xs", bufs=2) as xsp, \
                 tc.tile_pool(name="xw", bufs=3) as xwp, \
                 tc.tile_pool(name="os", bufs=2) as osp, \
                 tc.tile_pool(name="ow", bufs=3) as owp, \
                 tc.tile_pool(name="cpsum", bufs=4, space="PSUM") as cpp:

                for img in range(b_local):
                    # ---- x quantize (both ci halves) ----
                    xq_pad = []
                    for ci_half in range(2):
                        x_nat = xsp.tile([P, hw], F32, tag="xnat")
                        nc.sync.dma_start(
                            x_nat[:],
                            x_in[img,
                                 ci_half * P:(ci_half + 1) * P].rearrange(
                                "c h w -> c (h w)"))
                        qpad = xqp.tile([P, hwp], BF16, tag=f"qpad{ci_half}")
                        qpv = qpad[:].rearrange("p (r c) -> p r c", r=hp)
                        nc.gpsimd.memset(qpv[:, 0:hp:hp - 1, :], 0.0)
                        nc.gpsimd.memset(qpv[:, :, 0:wp:wp - 1], 0.0)
                        chunks, qfs = [], []
                        for xc in range(n_xc):
                            qf = xwp.tile([P, xc_pix], BF16, tag="xqf",
                                          name="qf")
                            chunks.append(
                                (x_nat[:, xc * xc_pix:(xc + 1) * xc_pix],
                                 qf[:]))
                            qfs.append(qf)
                        _emit_quant_group(nc, xwp, chunks, "x", BF16, consts)
                        for xc, qf in enumerate(qfs):
                            nc.scalar.copy(
                                qpv[:, xc * xc_rows + 1:
                                    (xc + 1) * xc_rows + 1, 1:1 + w],
                                qf[:].rearrange("p (r c) -> p r c",
                                                r=xc_rows))
                        xq_pad.append(qpad)

                    # ---- conv + out quantize ----
                    for co_half in range(2):
                        o_nat = osp.tile([P, hw], F32, tag="onat")
                        for pt in range(n_pt):
                            ps = cpp.tile([P, npix_t], F32, tag="cps")
                            k = 0
                            for ci_half in range(2):
                                qpv = xq_pad[ci_half][:].rearrange(
                                    "p (r c) -> p r c", r=hp)
                                for kh in range(3):
                                    for kw in range(3):
                                        khw = kh * 3 + kw
                                        rhs = qpv[:, pt * rpt + kh:
                                                  pt * rpt + kh + rpt,
                                                  kw:kw + w]
                                        nc.tensor.matmul(
                                            ps[:].rearrange(
                                                "p (r c) -> p r c", r=rpt),
                                            lhsT[ci_half][
                                                :, khw * COUT + co_half * P:
                                                khw * COUT + co_half * P + P],
                                            rhs,
                                            start=(k == 0), stop=(k == 17))
                                        k += 1
                            nc.scalar.activation(
                                o_nat[:, pt * npix_t:(pt + 1) * npix_t],
                                ps[:],
                                mybir.ActivationFunctionType.Identity,
                                bias=bias_sb[:, co_half:co_half + 1])

                        chunks, oqs = [], []
                        for (spt, cpix) in out_chunks:
                            oq = osp.tile([P, cpix], F32, tag="oq",
                                          name="oq")
                            chunks.append(
                                (o_nat[:, spt * npix_t:spt * npix_t + cpix],
                                 oq[:]))
                            oqs.append(oq)
                        _emit_quant_group(nc, owp, chunks, "o", F32, consts)
                        for (spt, cpix), oq in zip(out_chunks, oqs):
                            nc.sync.dma_start(
                                o_out[img, co_half * P:
                                      (co_half + 1) * P].rearrange(
                                    "c h w -> c (h w)")[
                                    :, spt * npix_t:spt * npix_t + cpix],
                                oq[:])
    nc.compile()
    return nc


_NC_CACHE = {}


def _get_nc(key):
    if key not in _NC_CACHE:
        _NC_CACHE[key] = build_kernel(*key)
    return _NC_CACHE[key]


def kernel(x, weight, bias):
    from concourse import bass_utils

    nc = _get_nc((B_LOCAL, H, W))
    in_maps = []
    for core in range(N_CORES):
        in_maps.append({
            "x": np.ascontiguousarray(x[core * B_LOCAL:(core + 1) * B_LOCAL]),
            "weight": np.ascontiguousarray(weight),
            "bias": np.ascontiguousarray(bias),
        })
    res = bass_utils.run_bass_kernel_spmd(
        nc, in_maps, core_ids=list(range(N_CORES)),
        trace=bool(int(os.environ.get("BFC_TRACE", "0"))),
    )
    out = np.concatenate([r["out"] for r in res.results], axis=0)
    kernel.last_exec_time_ns = res.exec_time_ns
    kernel.last_mean_exec_time_ns = res.mean_exec_time_ns
    kernel.last_trace = res.instructions_and_trace
    return out


kernel.last_exec_time_ns = None
